# revision 27
# baseline (speedup 1.0000x reference)
"""Trainium2 Bass kernel: CRF loss (nn_CRF_60112362275454).

Strategy (data-parallel over batch, 8 cores x 8 batch elems):
  - emit^T[tag, (s,b)] = Wdup^T @ features^T via PE, K=1024 tiled by 128.
    lhsT is W duplicated to 128 columns so PSUM rows 0-63 and 64-127 both
    hold emit^T (feeds the block-diagonal scan below).
  - Forward recurrence in LINEAR space: P_t = E^T (P_{t-1} * exp(emit_t))
    with E = exp(transitions); constant renorm P *= 2^-52 every 8 steps
    (exact power of two; fp32 range validated offline: |P| <= ~1e16).
  - Block-diagonal scan: stationary diag(E, E) [128,128]; state [128, 4]
    holds batch 0-3 on partitions 0-63 and batch 4-7 on partitions 64-127,
    so ONE matmul + ONE DVE mul advances all 8 batch elems per step.
  - Tag axis permuted (0<->32<->1 cycle) so EOS lands on partitions 0/64
    (per-step ACT snapshot) and BOS on 32/96 (init mask).
  - Gold path: host-prepared one-hot/count masks (index preprocessing of
    int inputs only); all f32 FLOPs on device.
  - Each core emits a partial loss scalar; host sums the 8 partials.
"""
import numpy as np
from contextlib import ExitStack

import concourse.bass as bass
import concourse.mybir as mybir
import concourse.tile as tile
from concourse.bass_utils import run_bass_kernel_spmd

S, B, D, T = 256, 64, 1024, 64
BOS, EOS, PAD = 0, 1, 2
NCORES = 8
BS = B // NCORES          # 8 batch elems per core
SB = S * BS               # 2048 (s,b) columns per core
R = 8                     # renorm cadence (steps)
RENORM = 2.0 ** -52       # exact power-of-two rescale
C_LOG = 52 * float(np.log(2.0))
CW = BS // 2              # 4 batch columns per block half
SC = S * CW               # 1024 scan columns per half
KT = D // 128             # 8 K-tiles
NCHUNK = 4                # emit column chunks
CHUNK = SB // NCHUNK      # 512

F32 = mybir.dt.float32
AF = mybir.ActivationFunctionType
ALU = mybir.AluOpType


def _papi(ap, plist):
    """AP with a custom [step,count] list on the same tensor/offset."""
    return bass.AP(ap.tensor, ap.offset, plist)


def _build_nc():
    nc = bass.Bass()
    # feat host-transposed to [D, S*BS]: 8KB contiguous HBM runs per row.
    feat = nc.dram_tensor("feat", [D, SB], F32, kind="ExternalInput")
    wt = nc.dram_tensor("wt", [D, 2 * T], F32, kind="ExternalInput")  # dup cols
    bias = nc.dram_tensor("bias", [2 * T, 1], F32, kind="ExternalInput")
    transp = nc.dram_tensor("transp", [T, T], F32, kind="ExternalInput")
    gmask = nc.dram_tensor("gmask", [T, SB], F32, kind="ExternalInput")
    c64 = nc.dram_tensor("c64", [T, T], F32, kind="ExternalInput")
    gcount = nc.dram_tensor("gcount", [T, 1], F32, kind="ExternalInput")
    pickmask = nc.dram_tensor("pickmask", [2, SC], F32, kind="ExternalInput")
    cw = nc.dram_tensor("cw", [2, CW], F32, kind="ExternalInput")
    out = nc.dram_tensor("out", [1, 1], F32, kind="ExternalOutput")

    with tile.TileContext(nc) as tc, ExitStack() as ctx:
        consts = ctx.enter_context(tc.tile_pool(name="consts", bufs=1))
        featp = ctx.enter_context(tc.tile_pool(name="featp", bufs=2))
        qp = ctx.enter_context(tc.tile_pool(name="qp", bufs=4))
        emitp = ctx.enter_context(tc.tile_pool(name="emitp", bufs=1, space="PSUM"))
        scanp = ctx.enter_context(tc.tile_pool(name="scanp", bufs=2, space="PSUM"))

        # ---- constants in ----
        wt_sb = consts.tile([128, KT * 128], F32, tag="wt")
        for k in range(KT):
            nc.sync.dma_start(wt_sb[:, k * 128:(k + 1) * 128],
                              wt[k * 128:(k + 1) * 128, :])
        b_sb = consts.tile([128, 1], F32, tag="bias")
        nc.sync.dma_start(b_sb[:], bias[:, :])
        tr_sb = consts.tile([128, T], F32, tag="tr")  # transitions stacked twice
        nc.sync.dma_start(tr_sb[0:T, :], transp[:, :])
        nc.sync.dma_start(tr_sb[T:2 * T, :], transp[:, :])
        gm_sb = consts.tile([T, SB], F32, tag="gmask")
        nc.sync.dma_start(gm_sb[:], gmask[:, :])
        c64_sb = consts.tile([T, T], F32, tag="c64")
        nc.sync.dma_start(c64_sb[:], c64[:, :])
        gc_sb = consts.tile([T, 1], F32, tag="gcount")
        nc.sync.dma_start(gc_sb[:], gcount[:, :])
        # pickmask/cw land on partitions 0 and 64
        pm_sb = consts.tile([128, SC], F32, tag="pickmask")
        nc.sync.dma_start(_papi(pm_sb[:], [[64 * SC, 2], [1, SC]]), pickmask[:, :])
        cw_sb = consts.tile([128, CW], F32, tag="cw")
        nc.sync.dma_start(_papi(cw_sb[:], [[64 * CW, 2], [1, CW]]), cw[:, :])

        # block-diagonal exp(transitions): diag(E, E) [128, 128]
        E2 = consts.tile([128, 128], F32, tag="E2")
        nc.vector.memset(E2[:], 0.0)
        nc.scalar.activation(E2[0:T, 0:T], tr_sb[0:T, :], AF.Exp)
        nc.scalar.activation(E2[T:2 * T, T:2 * T], tr_sb[T:2 * T, :], AF.Exp)
        ones_sb = consts.tile([128, 1], F32, tag="ones")
        nc.vector.memset(ones_sb[:], 1.0)
        # BOS one-hot on partitions 32 and 96 (permuted BOS rows per half)
        bos2 = consts.tile([128, 1], F32, tag="bos2")
        nc.vector.memset(bos2[:], 0.0)
        nc.vector.memset(bos2[32:33, 0:1], 1.0)
        nc.vector.memset(bos2[96:97, 0:1], 1.0)

        # ---- emit matmul (k outer; one 4-bank PSUM tile, 4 col-chunks) ----
        emit_ps = emitp.tile([128, SB], F32, tag="emit")
        for k in range(KT):
            ft = featp.tile([128, SB], F32, tag="feat")
            nc.sync.dma_start(ft[:], feat[k * 128:(k + 1) * 128, :])
            for n in range(NCHUNK):
                nc.tensor.matmul(emit_ps[:, n * CHUNK:(n + 1) * CHUNK],
                                 wt_sb[:, k * 128:(k + 1) * 128],
                                 ft[:, n * CHUNK:(n + 1) * CHUNK],
                                 start=(k == 0), stop=(k == KT - 1))

        # ---- exp(emit + b) into duplicated scan layout [128, S*CW] ----
        # rows 0-63: cols (t, b0..3); rows 64-127: cols (t, b4..7)
        expemit = consts.tile([128, SC], F32, tag="expemit")
        src = emit_ps[:].rearrange("p (t b) -> p t b", b=BS)
        dstv = expemit[:].rearrange("p (t c) -> p t c", c=CW)
        nc.scalar.activation(dstv[0:T, :, :], src[0:T, :, 0:CW],
                             AF.Exp, bias=b_sb[0:T, 0:1])
        nc.scalar.activation(dstv[T:2 * T, :, :], src[T:2 * T, :, CW:BS],
                             AF.Exp, bias=b_sb[T:2 * T, 0:1])

        # ---- gold partials (rows 0-63 of emit PSUM = full emit^T) ----
        goldpart = consts.tile([128, 8], F32, tag="goldpart")
        nc.vector.memset(goldpart[:], 0.0)
        sc = consts.tile([T, SB], F32, tag="sc")
        nc.vector.tensor_mul(sc[:], emit_ps[0:T, :], gm_sb[:])
        nc.vector.reduce_sum(goldpart[0:T, 0:1], sc[:], axis=mybir.AxisListType.X)
        sc64 = consts.tile([T, T], F32, tag="sc64")
        nc.vector.tensor_mul(sc64[:], tr_sb[0:T, :], c64_sb[:])
        nc.vector.reduce_sum(goldpart[0:T, 1:2], sc64[:], axis=mybir.AxisListType.X)
        nc.vector.tensor_mul(goldpart[0:T, 2:3], b_sb[0:T, :], gc_sb[:])

        # ---- scan ----
        hist = consts.tile([128, SC], F32, tag="hist")
        nc.vector.memset(hist[0:1, 0:CW], 1.0)   # t=0 never picked; avoid NaN*0
        nc.vector.memset(hist[T:T + 1, 0:CW], 1.0)
        prev = None
        for t in range(S):
            q = qp.tile([128, CW], F32, tag="q")
            lo = t * CW
            if t == 0:
                # q0[i,b] = delta(i==BOSp) * exp(emit0)[i,b] -> P0 via step mm
                nc.vector.tensor_mul(q[:], _papi(bos2[:], [[1, 128], [0, CW]]),
                                     expemit[:, lo:lo + CW])
            else:
                if t > 1 and (t - 1) % R == 0:
                    nc.vector.tensor_scalar_mul(prev[:], prev[:], RENORM)
                nc.vector.tensor_mul(q[:], prev[:], expemit[:, lo:lo + CW])
            ns = scanp.tile([128, CW], F32, tag="scan")
            nc.tensor.matmul(ns[:], E2[:], q[:], start=True, stop=True)
            if t >= 1:
                # snapshot EOS rows: partition 0 on ACT, partition 64 on DVE
                nc.scalar.activation(hist[0:1, lo:lo + CW], ns[0:1, :], AF.Copy)
                nc.vector.tensor_copy(hist[T:T + 1, lo:lo + CW], ns[T:T + 1, :])
            prev = ns

        # ---- final assembly ----
        pmul = consts.tile([128, SC], F32, tag="pmul")
        pick4 = consts.tile([128, CW], F32, tag="pick4")
        zrow = consts.tile([128, CW], F32, tag="zrow")
        z2 = consts.tile([128, CW], F32, tag="z2")
        nc.vector.memset(z2[:], 0.0)
        for r in (0, T):
            nc.vector.tensor_mul(pmul[r:r + 1, :], hist[r:r + 1, :],
                                 pm_sb[r:r + 1, :])
            # reduce over t (stride CW) for each b
            nc.vector.reduce_sum(
                pick4[r:r + 1, :],
                _papi(pmul[r:r + 1, :], [[SC, 1], [1, CW], [CW, S]]),
                axis=mybir.AxisListType.X)
            nc.scalar.activation(zrow[r:r + 1, :], pick4[r:r + 1, :], AF.Ln)
            nc.vector.tensor_add(z2[r:r + 1, :], zrow[r:r + 1, :],
                                 cw_sb[r:r + 1, :])
        # cvec[p] = zsum[p] - goldsum[p]; loss = ones^T cvec via one matmul
        cvec = consts.tile([128, 1], F32, tag="cvec")
        nc.vector.reduce_sum(cvec[:], z2[:], axis=mybir.AxisListType.X)
        gvec = consts.tile([128, 1], F32, tag="gvec")
        nc.vector.reduce_sum(gvec[:], goldpart[:], axis=mybir.AxisListType.X)
        dvec = consts.tile([128, 1], F32, tag="dvec")
        nc.vector.tensor_sub(dvec[:], cvec[:], gvec[:])
        loss_ps = emitp.tile([1, 1], F32, tag="emit", name="loss_ps")
        nc.tensor.matmul(loss_ps[:], ones_sb[:], dvec[:], start=True, stop=True)
        lossp = consts.tile([1, 1], F32, tag="lossp")
        nc.vector.tensor_copy(lossp[:], loss_ps[:])
        nc.sync.dma_start(out[:, :], lossp[:])

    # Raw Bass under TileContext skips two bacc legalization passes the NEFF
    # compiler requires: populating .instr bytes for extended-ISA insts, and
    # splitting >2 on_wait entries onto InstEventSemaphore (walrus rejects
    # "Too many sync wait commands" otherwise).
    mybir.codegen_inst_isa_subclasses(nc)
    import bass_rust
    bass_rust.generate_event_semaphores(nc)
    return nc


_CACHE = {}


def _get_nc():
    if "nc" not in _CACHE:
        _CACHE["nc"] = _build_nc()
    return _CACHE["nc"]


def _host_prep(features, tags, seq_lens, W, b, transitions):
    features = np.ascontiguousarray(np.asarray(features, dtype=np.float32))
    tags = np.asarray(tags).astype(np.int64)
    seq_lens = np.asarray(seq_lens).astype(np.int64)
    W = np.asarray(W, dtype=np.float32)
    bvec = np.asarray(b, dtype=np.float32)
    transitions = np.asarray(transitions, dtype=np.float32)

    # tag permutation sigma(old)=new: EOS->0 (hist snapshots on partitions
    # 0/64), BOS->32 (matmul base-partition constraint), 3-cycle 0->32->1->0.
    sigma = np.arange(T)
    sigma[EOS], sigma[BOS], sigma[32] = 0, 32, 1
    inv = np.argsort(sigma)
    Wt_p = np.ascontiguousarray(W[inv, :].T)                   # [D, T]
    wt_dup = np.ascontiguousarray(np.concatenate([Wt_p, Wt_p], axis=1))
    b_p = bvec[inv].reshape(T, 1)
    b_dup = np.ascontiguousarray(np.concatenate([b_p, b_p], axis=0))
    trans_p = np.ascontiguousarray(transitions[np.ix_(inv, inv)])

    pad_row = np.full((1, B), PAD, tags.dtype)
    nxt = np.concatenate([tags[1:], pad_row], axis=0)
    active = np.arange(S)[:, None] < seq_lens[None, :]          # s <= len-1
    tstar = seq_lens - 1
    wnum = (seq_lens - 2) // R

    in_maps = []
    for c in range(NCORES):
        bsl = slice(c * BS, (c + 1) * BS)
        # [S, BS, D] -> [D, S*BS] host transpose (DMA layout prep)
        f_c = np.ascontiguousarray(
            features[:, bsl, :].transpose(2, 0, 1).reshape(D, SB))
        tg = tags[:, bsl]
        nx = nxt[:, bsl]
        act = active[:, bsl].astype(np.float32)
        gm = np.zeros((T, SB), np.float32)
        cols = np.arange(SB).reshape(S, BS)
        gm[sigma[tg].ravel(), cols.ravel()] = act.ravel()
        c64m = np.zeros((T, T), np.float32)
        np.add.at(c64m, (sigma[tg].ravel(), sigma[nx].ravel()), act.ravel())
        gc = gm.sum(axis=1).reshape(T, 1).astype(np.float32)
        # pick one-hot per half: hist col layout is t*CW + (b mod CW)
        pm = np.zeros((2, SC), np.float32)
        ts_c = tstar[bsl]
        for bb in range(BS):
            pm[bb // CW, ts_c[bb] * CW + (bb % CW)] = 1.0
        cwv = (wnum[bsl].astype(np.float64) * C_LOG).astype(np.float32)
        cwv = np.ascontiguousarray(cwv.reshape(2, CW))
        in_maps.append({
            "feat": f_c, "wt": wt_dup, "bias": b_dup, "transp": trans_p,
            "gmask": gm, "c64": c64m, "gcount": gc, "pickmask": pm, "cw": cwv,
        })
    return in_maps


def kernel(features, tags, seq_lens, W, b, transitions):
    in_maps = _host_prep(features, tags, seq_lens, W, b, transitions)
    nc = _get_nc()
    res = run_bass_kernel_spmd(nc, in_maps, list(range(NCORES)))
    total = np.float64(0.0)
    for r in res.results:
        total += np.float64(np.asarray(r["out"]).reshape(-1)[0])
    return np.array(total, dtype=np.float32)
